# revision 1
# baseline (speedup 1.0000x reference)
"""nn_BitConv: ternary 3x3 conv (stride 1, pad 1) + BatchNorm(eval) + SiLU
on 8 Trainium2 NeuronCores, data-parallel over the batch dimension.

Strategy
--------
Host (numpy, negligible cost): ternarize the weight exactly like the
reference (scale = 1/median|w|, w_q = clamp(round(w*scale))/scale) and keep
only the integer part t in {-1,0,+1} (exact in bf16); fold the 1/scale
factor and the BatchNorm affine into a single per-output-channel
scale/shift (a, b). Pre-transpose the weight into the tensor-engine
stationary layout and zero-pad x to 58x58 / cast to bf16 (products with
ternary weights are exact; only the bf16 rounding of x itself contributes
error, ~1e-3 relative on the conv output).

Device (per core, 4 images): the 3x3 conv is 9 shifted matmuls x 2
C1-chunks of K=128 accumulated in PSUM. For each image, C2-chunk (2x128)
and 8-row output block (7 per image), 18 matmuls of [K=128, M=128] x
[128, N=8*56=448] accumulate one PSUM tile; a single ScalarE activation
applies Silu(a*z + b) fused, then the tile is DMA'd out. 1008 back-to-back
matmuls keep the PE warm; ACT/DMA run concurrently. Measured ~183 us per
core, ~ the bf16 PE roofline for the 14.8 GFLOP/core conv.
"""
import numpy as np
import ml_dtypes
import concourse.bass as bass
from concourse import mybir
from concourse.bass_utils import run_bass_kernel_spmd
from concourse.tile import TileContext
from concourse.vector_clock import ScopedClock

BF16 = mybir.dt.bfloat16
F32 = mybir.dt.float32
NP_BF16 = ml_dtypes.bfloat16

N_CORES = 8
B, C, H, W = 32, 256, 56, 56
B_LOC = B // N_CORES
HP, WP = H + 2, W + 2
RB = 8            # output rows per PSUM tile (N = 8*56 = 448 <= 512)
NRB = H // RB


class _SplitDrainTC(TileContext):
    """This walrus build allows a single sync wait on the SP CTRL (Drain)
    instruction; split the Tile tail drain's waits across extra drains."""

    def _drain_and_barrier(self, tick_clock, wait_clock):
        drain_inst = self.nc.sync.drain()
        wait_clock.add_sem_waits(
            drain_inst.ins, ScopedClock({None: tick_clock.global_clock})
        )
        si = drain_inst.ins.sync_info
        waits = list(si.on_wait or []) if si is not None else []
        if len(waits) > 1:
            si.on_wait = waits[:1]
            for k in range(1, len(waits)):
                d2 = self.nc.sync.drain()
                si2 = d2.ins.sync_info
                if si2 is None:
                    d2.ins.sync_info = mybir.SyncInfo(
                        on_wait=[waits[k]], on_update=[]
                    )
                else:
                    si2.on_wait = [waits[k]]
        self.nc.all_engine_barrier()
        assert self.sems is not None
        popped = self.nc._tile_sem_poison_stack.pop()
        assert popped is self._sem_poison
        self.nc.clear_and_free_semaphores(list(self.sems.allocated().values()))
        self.nc.all_engine_barrier()


def split_sync_waits(nc, limit=1):
    """Hoist excess per-instruction sem waits onto same-engine nops (this
    walrus build allows only `limit` sync waits per instruction)."""
    builders = {
        mybir.EngineType.PE: nc.tensor,
        mybir.EngineType.Activation: nc.scalar,
        mybir.EngineType.DVE: nc.vector,
        mybir.EngineType.Pool: nc.gpsimd,
        mybir.EngineType.SP: nc.sync,
    }
    n_split = 0
    for f in nc.m.functions:
        for bb in f.blocks:
            insts = bb.instructions
            idx = 0
            while idx < len(insts):
                inst = insts[idx]
                si = inst.sync_info
                waits = list(si.on_wait) if (si is not None and si.on_wait) else []
                if len(waits) <= limit:
                    idx += 1
                    continue
                eng = inst.engine
                if eng not in builders:
                    raise RuntimeError(
                        f"split_sync_waits: no builder for engine {eng} "
                        f"on {inst.name} ({type(inst).__name__})"
                    )
                si.on_wait = waits[-limit:]
                carriers = []
                for w in waits[:-limit]:
                    nop = builders[eng].nop(nofuse=True)
                    ci = nop.ins
                    tail_bb = nc.cur_bb.bb
                    assert tail_bb.instructions[-1] is ci
                    tail_bb.instructions.pop()
                    ci.sync_info = mybir.SyncInfo(on_wait=[w], on_update=[])
                    carriers.append(ci)
                for k, ci in enumerate(carriers):
                    insts.insert(idx + k, ci)
                n_split += 1
                idx += len(carriers) + 1
    return n_split


def build_nc(b_loc=B_LOC, repeats=1):
    nc = bass.Bass()
    xp_d = nc.dram_tensor("xp", [b_loc, 2, 128, HP, WP], BF16, kind="ExternalInput")
    wp_d = nc.dram_tensor("wp", [2, 128, 9, 2, 128], BF16, kind="ExternalInput")
    ab_d = nc.dram_tensor("ab", [2, 128, 2], F32, kind="ExternalInput")
    out_d = nc.dram_tensor("out", [b_loc, 2, 128, H, W], F32, kind="ExternalOutput")

    with _SplitDrainTC(nc) as tc:
        with (
            tc.tile_pool(name="consts", bufs=1) as consts,
            tc.tile_pool(name="xpool", bufs=1) as xpool,
            tc.tile_pool(name="psum", bufs=8, space="PSUM") as psum,
            tc.tile_pool(name="opool", bufs=4) as opool,
        ):
            w_sb = []
            for i in range(2):
                w = consts.tile([128, 9, 2, 128], BF16, tag=f"w{i}")
                nc.sync.dma_start(w[:], wp_d[i])
                w_sb.append(w)
            a_sb, b_sb = [], []
            for j in range(2):
                a = consts.tile([128, 1], F32, tag=f"a{j}")
                nc.sync.dma_start(a[:], ab_d[j, :, 0:1])
                a_sb.append(a)
                bt = consts.tile([128, 1], F32, tag=f"b{j}")
                nc.sync.dma_start(bt[:], ab_d[j, :, 1:2])
                b_sb.append(bt)
            x_sb = [[None] * 2 for _ in range(b_loc)]
            for n in range(b_loc):
                for i in range(2):
                    xt = xpool.tile([128, HP, WP], BF16, tag=f"x{n}_{i}")
                    nc.sync.dma_start(xt[:], xp_d[n, i])
                    x_sb[n][i] = xt

            for _rep in range(repeats):
                for n in range(b_loc):
                    for j in range(2):
                        for r in range(NRB):
                            ps = psum.tile([128, RB, W], F32, tag="ps")
                            idx = 0
                            for ky in range(3):
                                for kx in range(3):
                                    for i in range(2):
                                        nc.tensor.matmul(
                                            ps[:],
                                            w_sb[i][:, ky * 3 + kx, j, :],
                                            x_sb[n][i][
                                                :,
                                                r * RB + ky : r * RB + ky + RB,
                                                kx : kx + W,
                                            ],
                                            start=(idx == 0),
                                            stop=(idx == 17),
                                        )
                                        idx += 1
                            o = opool.tile([128, RB, W], F32, tag="o")
                            nc.scalar.activation(
                                o[:], ps[:],
                                mybir.ActivationFunctionType.Silu,
                                bias=b_sb[j][:], scale=a_sb[j][:],
                            )
                            nc.sync.dma_start(
                                out_d[n, j, :, r * RB : r * RB + RB, :], o[:]
                            )
    split_sync_waits(nc)
    return nc


def preprocess(x, weight, gamma, beta, running_mean, running_var):
    """Host-side prep: ternarize, fold BN + ternary scale, pad/pack/cast."""
    x = np.asarray(x, dtype=np.float32)
    w = np.asarray(weight, dtype=np.float32)
    gamma = np.asarray(gamma, dtype=np.float32)
    beta = np.asarray(beta, dtype=np.float32)
    rm = np.asarray(running_mean, dtype=np.float32)
    rv = np.asarray(running_var, dtype=np.float32)

    s = np.float32(np.median(np.abs(w)))
    s_c = np.maximum(s, np.float32(1e-5))        # 1/scale of the reference
    scale = np.float32(1.0) / s_c
    t = np.clip(np.round(w * scale), -1.0, 1.0).astype(np.float32)

    inv = gamma / np.sqrt(rv + np.float32(1e-5))
    a = (s_c * inv).astype(np.float32)
    b = (beta - rm * inv).astype(np.float32)

    # [C2, C1, 3, 3] -> [i(c1 chunk), c1in, tap, j(c2 chunk), c2in]
    wp = (
        t.reshape(2, 128, 2, 128, 3, 3)
        .transpose(2, 3, 4, 5, 0, 1)
        .reshape(2, 128, 9, 2, 128)
        .astype(NP_BF16)
    )
    ab = np.stack([a.reshape(2, 128), b.reshape(2, 128)], axis=-1).astype(
        np.float32
    )

    xp = np.zeros((B, 2, 128, HP, WP), dtype=NP_BF16)
    xp[:, :, :, 1 : H + 1, 1 : W + 1] = x.reshape(B, 2, 128, H, W).astype(NP_BF16)
    return xp, wp, ab


_NC_CACHE = {}


def get_nc(repeats=1):
    if repeats not in _NC_CACHE:
        _NC_CACHE[repeats] = build_nc(B_LOC, repeats=repeats)
    return _NC_CACHE[repeats]


def make_in_maps(xp, wp, ab):
    return [
        {
            "xp": np.ascontiguousarray(xp[c * B_LOC : (c + 1) * B_LOC]),
            "wp": wp,
            "ab": ab,
        }
        for c in range(N_CORES)
    ]


def kernel(x, weight, gamma, beta, running_mean, running_var):
    xp, wp, ab = preprocess(x, weight, gamma, beta, running_mean, running_var)
    nc = get_nc()
    res = run_bass_kernel_spmd(nc, make_in_maps(xp, wp, ab), list(range(N_CORES)))
    return np.concatenate(
        [r["out"].reshape(B_LOC, C, H, W) for r in res.results], axis=0
    )


# revision 2
# speedup vs baseline: 1.0810x; 1.0810x over previous
"""nn_BitConv: ternary 3x3 conv (stride 1, pad 1) + BatchNorm(eval) + SiLU
on 8 Trainium2 NeuronCores, data-parallel over the batch dimension.

Strategy
--------
Host (numpy, negligible cost): ternarize the weight exactly like the
reference (scale = 1/median|w|, w_q = clamp(round(w*scale))/scale) and keep
only the integer part t in {-1,0,+1} (exact in bf16); fold the 1/scale
factor and the BatchNorm affine into a single per-output-channel
scale/shift (a, b). Pre-transpose the weight into the tensor-engine
stationary layout and zero-pad x to 58x58 / cast to bf16 (products with
ternary weights are exact; only the bf16 rounding of x itself contributes
error, ~1e-3 relative on the conv output).

Device (per core, 4 images): the 3x3 conv is 9 shifted matmuls x 2
C1-chunks of K=128 accumulated in PSUM. For each image, C2-chunk (2x128)
and 8-row output block (7 per image), 18 matmuls of [K=128, M=128] x
[128, N=8*56=448] accumulate one PSUM tile; a single ScalarE activation
applies Silu(a*z + b) fused, then the tile is DMA'd out. 1008 back-to-back
matmuls keep the PE warm; ACT/DMA run concurrently. Measured ~183 us per
core, ~ the bf16 PE roofline for the 14.8 GFLOP/core conv.
"""
import numpy as np
import ml_dtypes
import concourse.bass as bass
from concourse import mybir
from concourse.bass_utils import run_bass_kernel_spmd
from concourse.tile import TileContext
from concourse.vector_clock import ScopedClock

BF16 = mybir.dt.bfloat16
F32 = mybir.dt.float32
NP_BF16 = ml_dtypes.bfloat16

N_CORES = 8
B, C, H, W = 32, 256, 56, 56
B_LOC = B // N_CORES
HP, WP = H + 2, W + 2
RB = 8            # output rows per PSUM tile (N = 8*56 = 448 <= 512)
NRB = H // RB


class _SplitDrainTC(TileContext):
    """This walrus build allows a single sync wait on the SP CTRL (Drain)
    instruction; split the Tile tail drain's waits across extra drains."""

    def _drain_and_barrier(self, tick_clock, wait_clock):
        drain_inst = self.nc.sync.drain()
        wait_clock.add_sem_waits(
            drain_inst.ins, ScopedClock({None: tick_clock.global_clock})
        )
        si = drain_inst.ins.sync_info
        waits = list(si.on_wait or []) if si is not None else []
        if len(waits) > 1:
            si.on_wait = waits[:1]
            for k in range(1, len(waits)):
                d2 = self.nc.sync.drain()
                si2 = d2.ins.sync_info
                if si2 is None:
                    d2.ins.sync_info = mybir.SyncInfo(
                        on_wait=[waits[k]], on_update=[]
                    )
                else:
                    si2.on_wait = [waits[k]]
        self.nc.all_engine_barrier()
        assert self.sems is not None
        popped = self.nc._tile_sem_poison_stack.pop()
        assert popped is self._sem_poison
        self.nc.clear_and_free_semaphores(list(self.sems.allocated().values()))
        self.nc.all_engine_barrier()


def split_sync_waits(nc, limit=1):
    """Hoist excess per-instruction sem waits onto same-engine nops (this
    walrus build allows only `limit` sync waits per instruction)."""
    builders = {
        mybir.EngineType.PE: nc.tensor,
        mybir.EngineType.Activation: nc.scalar,
        mybir.EngineType.DVE: nc.vector,
        mybir.EngineType.Pool: nc.gpsimd,
        mybir.EngineType.SP: nc.sync,
    }
    n_split = 0
    for f in nc.m.functions:
        for bb in f.blocks:
            insts = bb.instructions
            idx = 0
            while idx < len(insts):
                inst = insts[idx]
                si = inst.sync_info
                waits = list(si.on_wait) if (si is not None and si.on_wait) else []
                if len(waits) <= limit:
                    idx += 1
                    continue
                eng = inst.engine
                if eng not in builders:
                    raise RuntimeError(
                        f"split_sync_waits: no builder for engine {eng} "
                        f"on {inst.name} ({type(inst).__name__})"
                    )
                si.on_wait = waits[-limit:]
                carriers = []
                for w in waits[:-limit]:
                    nop = builders[eng].nop(nofuse=True)
                    ci = nop.ins
                    tail_bb = nc.cur_bb.bb
                    assert tail_bb.instructions[-1] is ci
                    tail_bb.instructions.pop()
                    ci.sync_info = mybir.SyncInfo(on_wait=[w], on_update=[])
                    carriers.append(ci)
                for k, ci in enumerate(carriers):
                    insts.insert(idx + k, ci)
                n_split += 1
                idx += len(carriers) + 1
    return n_split


def build_nc(b_loc=B_LOC, repeats=1):
    nc = bass.Bass()
    xp_d = nc.dram_tensor("xp", [b_loc, 2, 128, HP, WP], BF16, kind="ExternalInput")
    wp_d = nc.dram_tensor("wp", [2, 128, 9, 2, 128], BF16, kind="ExternalInput")
    ab_d = nc.dram_tensor("ab", [2, 128, 2], F32, kind="ExternalInput")
    out_d = nc.dram_tensor("out", [b_loc, 2, 128, H, W], F32, kind="ExternalOutput")

    with _SplitDrainTC(nc) as tc:
        with (
            tc.tile_pool(name="consts", bufs=1) as consts,
            tc.tile_pool(name="xpool", bufs=1) as xpool,
            tc.tile_pool(name="psum", bufs=8, space="PSUM") as psum,
            tc.tile_pool(name="opool", bufs=4) as opool,
        ):
            w_sb = []
            for i in range(2):
                w = consts.tile([128, 9, 2, 128], BF16, tag=f"w{i}")
                nc.sync.dma_start(w[:], wp_d[i])
                w_sb.append(w)
            a_sb, b_sb = [], []
            for j in range(2):
                a = consts.tile([128, 1], F32, tag=f"a{j}")
                nc.sync.dma_start(a[:], ab_d[j, :, 0:1])
                a_sb.append(a)
                bt = consts.tile([128, 1], F32, tag=f"b{j}")
                nc.sync.dma_start(bt[:], ab_d[j, :, 1:2])
                b_sb.append(bt)
            x_sb = [[None] * 2 for _ in range(b_loc)]
            for n in range(b_loc):
                for i in range(2):
                    xt = xpool.tile([128, HP, WP], BF16, tag=f"x{n}_{i}")
                    nc.sync.dma_start(xt[:], xp_d[n, i])
                    x_sb[n][i] = xt

            for _rep in range(repeats):
                for n in range(b_loc):
                    for j in range(2):
                        for r in range(NRB):
                            ps = psum.tile([128, RB, W], F32, tag="ps")
                            idx = 0
                            for ky in range(3):
                                for kx in range(3):
                                    for i in range(2):
                                        nc.tensor.matmul(
                                            ps[:],
                                            w_sb[i][:, ky * 3 + kx, j, :],
                                            x_sb[n][i][
                                                :,
                                                r * RB + ky : r * RB + ky + RB,
                                                kx : kx + W,
                                            ],
                                            start=(idx == 0),
                                            stop=(idx == 17),
                                        )
                                        idx += 1
                            o = opool.tile([128, RB, W], F32, tag="o")
                            nc.scalar.activation(
                                o[:], ps[:],
                                mybir.ActivationFunctionType.Silu,
                                bias=b_sb[j][:], scale=a_sb[j][:],
                            )
                            nc.sync.dma_start(
                                out_d[n, j, :, r * RB : r * RB + RB, :], o[:]
                            )
    split_sync_waits(nc)
    return nc


def preprocess(x, weight, gamma, beta, running_mean, running_var):
    """Host-side prep: ternarize, fold BN + ternary scale, pad/pack/cast."""
    x = np.asarray(x, dtype=np.float32)
    w = np.asarray(weight, dtype=np.float32)
    gamma = np.asarray(gamma, dtype=np.float32)
    beta = np.asarray(beta, dtype=np.float32)
    rm = np.asarray(running_mean, dtype=np.float32)
    rv = np.asarray(running_var, dtype=np.float32)

    s = np.float32(np.median(np.abs(w)))
    s_c = np.maximum(s, np.float32(1e-5))        # 1/scale of the reference
    scale = np.float32(1.0) / s_c
    t = np.clip(np.round(w * scale), -1.0, 1.0).astype(np.float32)

    inv = gamma / np.sqrt(rv + np.float32(1e-5))
    a = (s_c * inv).astype(np.float32)
    b = (beta - rm * inv).astype(np.float32)

    # [C2, C1, 3, 3] -> [i(c1 chunk), c1in, tap, j(c2 chunk), c2in]
    wp = (
        t.reshape(2, 128, 2, 128, 3, 3)
        .transpose(2, 3, 4, 5, 0, 1)
        .reshape(2, 128, 9, 2, 128)
        .astype(NP_BF16)
    )
    ab = np.stack([a.reshape(2, 128), b.reshape(2, 128)], axis=-1).astype(
        np.float32
    )

    xp = np.zeros((B, 2, 128, HP, WP), dtype=NP_BF16)
    xp[:, :, :, 1 : H + 1, 1 : W + 1] = x.reshape(B, 2, 128, H, W).astype(NP_BF16)
    return xp, wp, ab


_NC_CACHE = {}


def get_nc(repeats=1):
    if repeats not in _NC_CACHE:
        _NC_CACHE[repeats] = build_nc(B_LOC, repeats=repeats)
    return _NC_CACHE[repeats]


def make_in_maps(xp, wp, ab):
    # dim-0 slices of a C-contiguous array are already contiguous
    return [
        {"xp": xp[c * B_LOC : (c + 1) * B_LOC], "wp": wp, "ab": ab}
        for c in range(N_CORES)
    ]


def kernel(x, weight, gamma, beta, running_mean, running_var):
    xp, wp, ab = preprocess(x, weight, gamma, beta, running_mean, running_var)
    nc = get_nc()
    res = run_bass_kernel_spmd(nc, make_in_maps(xp, wp, ab), list(range(N_CORES)))
    return np.concatenate(
        [r["out"].reshape(B_LOC, C, H, W) for r in res.results], axis=0
    )
